# revision 41
# baseline (speedup 1.0000x reference)
"""Trainium2 Bass kernel for nn_CropRoi (FPN ROI crop / roi-align style).

Contract: kernel(**inputs) takes the FULL inputs (p2..p5 feature pyramid,
proposals [1024, 7]) and returns the FULL output [1024, 256, 14, 14] f32.

Strategy (final)
----------------
Routing/index math and the pixel gather live on the host; the device is a
streaming pipeline.  HBM traffic is ~16.3 MB/core (vs 27.2 MB in the v2
baseline: patch 2.4 MB + factor streams 1.1 MB + output 12.85 MB):

  * separable bilinear weights: per-pixel 14-value row/col factors
    (wyab/wxseg, 56+56 B per pixel-row) instead of dense 784 B blocks;
    each segment's [128, 392] 2D weight block is built on device by
    broadcast outer-product ops (BG segments batched per instruction,
    2 of 3 batches on DVE, 1 of 3 on the otherwise-idle GpSimd).
  * overlap-aware global pairing: proposals are paired to maximize
    shared feature-pixel footprint (union gather) and minimize 32-px
    quad roundup; pairs are sorted by quad count and snake-dealt to the
    8 cores, so all cores share one program template by construction
    (position k capacity = block max) -> R = 128 output rows/core.

Per template instance (one pair, 2 proposals):
  for each quad-aligned segment (slot, part0, cnt):
    build: w2d[p, b*196+i*14+j] = wyab[p, t, b, i] * wxseg[p, t, b, j]
    TensorE: ps[0:392]   += patch_even(slot).T @ w2d   (bank 0/2)
             ps[512:904] += patch_odd(slot).T  @ w2d   (bank 1/3)
    (pixels outside the pair get zero factor rows -> no cross-talk
     between pairs sharing a slot; contraction is always 128 partitions)
Two instances share one 4-bank PSUM tile; a single strided copy (ACT for
~5 of 8 groups, DVE else) drains both pairs f32->f16 into the stage.
Instances are emitted in reverse template order (small pairs first) so
the pipeline fills fast; the factor streams are laid out in HBM in
emission order so build batches are consumption-contiguous.  Chunked
HWDGE stores stage -> out [128, R, 2, 196] f16; host converts f16 ->
f32, de-interleaves channels and scatters rows back to proposal order.
"""

import os
import sys

for _p in ("/opt/trn_rl_repo",):
    if os.path.isdir(_p) and _p not in sys.path:
        sys.path.insert(0, _p)

import numpy as np

import concourse.bass as bass
import concourse.bacc as bacc
import concourse.mybir as mybir
from concourse.tile import TileContext
from concourse import bass_utils

# ---------------------------------------------------------------- constants
IMG = 1024
CS = 14
TT = CS * CS  # 196
STRIDES = (4, 8, 16, 32)
BASE_SIZES = (8.0, 16.0, 32.0, 64.0)
B = 2
C = 256
NPROP = 1024
NCORES = 8
QUAD = 32
QPS = 4  # quads per slot

HWL = [IMG // s for s in STRIDES]
NPIXL = [B * h * h for h in HWL]
LEVEL_OFF = np.cumsum([0] + NPIXL)[:4]

_F32 = mybir.dt.float32
_F16 = mybir.dt.float16

CHUNK_ROWS = 12    # output rows per stage tile / out-DMA
LOAD_SLOTS = 8     # slots per patch-load chunk
LOAD_SEGS = 24     # segments per factor-load chunk
BG = 6             # segments per batched weight-build op
GP_BUILD = 3       # every GP_BUILD'th build batch runs on GpSimd
DVE_DRAIN = 4      # every DVE_DRAIN'th drain group runs on DVE (rest ACT)


# ---------------------------------------------------------------- planner
def _plan_proposals(proposals):
    """Per-proposal gather indices + separable per-pixel weight factors."""
    pr = np.asarray(proposals, dtype=np.float32)
    n = pr.shape[0]
    bi = pr[:, 0].astype(np.int32)
    x0, y0, x1, y1 = pr[:, 1], pr[:, 2], pr[:, 3], pr[:, 4]
    sizes = np.sqrt((x1 - x0) * (y1 - y0))
    base = np.asarray(BASE_SIZES, np.float32)
    lvl = np.argmin(np.abs(sizes[:, None] - base[None, :]), axis=1)
    grid = (np.arange(CS, dtype=np.float32) / np.float32(CS - 1))
    ar = np.arange(CS)
    plans = []
    for i in range(n):
        l = int(lvl[i])
        H = HWL[l]
        s = np.float32(1.0 / STRIDES[l])
        ys = y0[i] * s + (y1[i] - y0[i]) * s * grid
        xs = x0[i] * s + (x1[i] - x0[i]) * s * grid
        yf = np.floor(ys)
        xf = np.floor(xs)
        ly = ys - yf
        lx = xs - xf
        yi0 = np.clip(yf.astype(np.int64), 0, H - 1)
        yi1 = np.clip(yi0 + 1, 0, H - 1)
        xi0 = np.clip(xf.astype(np.int64), 0, H - 1)
        xi1 = np.clip(xi0 + 1, 0, H - 1)
        ylo = int(yi0.min())
        hp = int(yi1.max()) - ylo + 1
        xlo = int(xi0.min())
        wp = int(xi1.max()) - xlo + 1
        wyrow = np.zeros((hp, CS), np.float32)
        np.add.at(wyrow, (yi0 - ylo, ar), 1.0 - ly)
        np.add.at(wyrow, (yi1 - ylo, ar), ly)
        wxcol = np.zeros((wp, CS), np.float32)
        np.add.at(wxcol, (xi0 - xlo, ar), 1.0 - lx)
        np.add.at(wxcol, (xi1 - xlo, ar), lx)
        pbase = int(LEVEL_OFF[l]) + int(bi[i]) * H * H
        idx = (pbase + (ylo + np.arange(hp))[:, None] * H
               + (xlo + np.arange(wp))[None, :]).reshape(-1).astype(np.int32)
        wy_px = np.repeat(wyrow, wp, axis=0).astype(np.float16)   # [npx,14]
        wx_px = np.tile(wxcol, (hp, 1)).astype(np.float16)        # [npx,14]
        plans.append((idx, wy_px, wx_px))
    return plans


def _pair_overlap(plans):
    """Greedy pairing maximizing footprint overlap (minus quad waste).
    Returns pairs: {q, nu, pids: [pa, pb], uidx, pos: per-prop positions
    into the union stream}."""
    npx = np.array([len(p[0]) for p in plans])
    n = len(npx)
    order = sorted(range(n), key=lambda i: -npx[i])
    idxsets = [frozenset(plans[i][0].tolist()) for i in range(n)]
    used = [False] * n
    pairs = []
    for ai in range(n):
        if used[ai]:
            continue
        used[ai] = True
        ia = order[ai]
        sa = idxsets[ia]
        na = len(sa)
        best = None
        for bi in range(ai + 1, n):
            if used[bi]:
                continue
            ib = order[bi]
            inter = len(sa & idxsets[ib])
            un = na + len(idxsets[ib]) - inter
            waste = (-un) % QUAD
            score = inter - 0.5 * waste
            if best is None or score > best[0]:
                best = (score, bi)
        assert best is not None, "odd proposal count unsupported"
        _, bi = best
        used[bi] = True
        ib = order[bi]
        idx_a = plans[ia][0]
        idx_b = plans[ib][0]
        pos = {int(p): k for k, p in enumerate(idx_a)}
        uidx = list(idx_a)
        pos_b = np.empty(len(idx_b), np.int64)
        for k, p in enumerate(idx_b):
            p = int(p)
            j = pos.get(p)
            if j is None:
                j = len(uidx)
                uidx.append(p)
                pos[p] = j
            pos_b[k] = j
        nu = len(uidx)
        q = (nu + QUAD - 1) // QUAD
        pairs.append({
            "q": q, "nu": nu,
            "pids": [ia, ib],
            "uidx": np.asarray(uidx, np.int32),
            "pos": [np.arange(len(idx_a)), pos_b],
        })
    return pairs


def _deal(pairs):
    """Snake-deal pairs (sorted by q desc) to cores.  Template position k
    gets capacity q = max of its 8-pair block; every core holds one pair
    per position, so all cores share the program structure exactly."""
    pairs = sorted(pairs, key=lambda g: -g["q"])
    npos = (len(pairs) + NCORES - 1) // NCORES
    template = []
    core_groups = [[] for _ in range(NCORES)]
    for k in range(npos):
        block = pairs[k * NCORES:(k + 1) * NCORES]
        template.append((block[0]["q"], 2))
        order = range(NCORES) if k % 2 == 0 else range(NCORES - 1, -1, -1)
        for j, c in enumerate(order):
            core_groups[c].append(block[j] if j < len(block) else None)
    return template, core_groups


def _place(template):
    """Quad placement per template instance: best-fit small pairs into
    free intra-slot runs, big pairs across fresh slots."""
    free = []  # (slot, q0, len) free runs, each within one slot
    nslots = 0
    inst_segs = []
    for (q, npr) in template:
        quads = []
        if q <= QPS:
            best = None
            for ri, (sl, q0, ln) in enumerate(free):
                if ln >= q and (best is None or ln < best[1]):
                    best = (ri, ln)
            if best is not None:
                ri, _ = best
                sl, q0, ln = free[ri]
                quads.append((sl, q0, q))
                if ln > q:
                    free[ri] = (sl, q0 + q, ln - q)
                else:
                    free.pop(ri)
            else:
                sl = nslots
                nslots += 1
                quads.append((sl, 0, q))
                if q < QPS:
                    free.append((sl, q, QPS - q))
        else:
            left = q
            while left > 0:
                sl = nslots
                nslots += 1
                ln = min(QPS, left)
                quads.append((sl, 0, ln))
                left -= ln
            if q % QPS:
                free.append((nslots - 1, q % QPS, QPS - q % QPS))
        segs = []
        off = 0
        for (sl, q0, ln) in quads:
            segs.append((sl, q0 * QUAD, ln * QUAD, off))
            off += ln * QUAD
        inst_segs.append(segs)
    g_row = []
    r = 0
    for (q, npr) in template:
        g_row.append(r)
        r += npr
    R = r
    chunks = []  # (g0, ng, r0, rows)
    i = 0
    first = True
    while i < len(template):
        j = i
        rows = 0
        # instances emit in reverse chunk order, so the FIRST chunk here
        # is the LAST emitted: keep it tiny (2 big pairs) so the final
        # drain + store tail is short
        cap = 4 if first else CHUNK_ROWS
        while j < len(template) and rows + template[j][1] <= cap:
            rows += template[j][1]
            j += 1
        chunks.append((i, j - i, g_row[i], rows))
        i = j
        first = False
    seg_off = np.cumsum([0] + [len(s) for s in inst_segs])
    return nslots, R, inst_segs, g_row, chunks, seg_off


def _emission(template, inst_segs, chunks, seg_off):
    """Emission order: chunks in reverse (small pairs first, the tiny
    first chunk of big pairs last); instances within a chunk in reverse.
    The factor streams are laid out in HBM by emission position so build
    batches are consumption-contiguous."""
    nch = len(chunks)
    chunk_order = list(range(nch - 1, -1, -1))
    gi_order = []
    for ci_ in chunk_order:
        g0, ng, r0, rows = chunks[ci_]
        gi_order.extend(range(g0 + ng - 1, g0 - 1, -1))
    T = int(seg_off[-1])
    pos_of_seg = np.zeros(T, np.int64)
    pos = 0
    for gi in gi_order:
        for si in range(len(inst_segs[gi])):
            pos_of_seg[int(seg_off[gi]) + si] = pos
            pos += 1
    return gi_order, pos_of_seg


# ---------------------------------------------------------------- device
def build_bass_program(layout):
    (template, S, R, T, inst_segs, g_row, chunks, seg_off,
     gi_order, pos_of_seg) = layout

    def _chunk_sizes(total, step):
        sizes = []
        left = total
        while left > 0:
            t = min(step, left)
            sizes.append(t)
            left -= t
        return sizes

    # small first patch chunk so the first matmuls start sooner
    psizes = ([min(4, S)] + _chunk_sizes(S - min(4, S), LOAD_SLOTS)
              if S > 4 else [S])
    pstarts = np.cumsum([0] + psizes)[:-1]
    # factor streams are laid out by emission position -> chunk first-use
    # is just its start position
    wsizes = _chunk_sizes(T, LOAD_SEGS)
    wstarts = np.cumsum([0] + wsizes)[:-1]
    seg_slot = [None] * T
    for gi, segs in enumerate(inst_segs):
        for si, (sl, part0, cnt, pixoff) in enumerate(segs):
            seg_slot[int(pos_of_seg[int(seg_off[gi]) + si])] = sl
    p_first = []
    for a, ln in zip(pstarts, psizes):
        uses = [pp for pp, sl in enumerate(seg_slot) if a <= sl < a + ln]
        p_first.append(min(uses) if uses else 0)
    w_first = [int(a) for a in wstarts]

    nc = bacc.Bacc("TRN2", target_bir_lowering=False, num_swdge_queues=2)
    patch_d = nc.dram_tensor("patch", [128, S * C], _F16, kind="ExternalInput")
    wyab_d = nc.dram_tensor("wyab", [128, T * 2 * CS], _F16,
                            kind="ExternalInput")
    wxseg_d = nc.dram_tensor("wxseg", [128, T * 2 * CS], _F16,
                             kind="ExternalInput")
    # partition-major output: out[p, r, h, t] holds channel 2p+h of row r.
    out_d = nc.dram_tensor("out", [128, R * 2 * TT], _F16,
                           kind="ExternalOutput")

    with TileContext(nc) as tc:
        with tc.tile_pool(name="in", bufs=1) as ipool, \
             tc.tile_pool(name="w2d", bufs=4) as wpool, \
             tc.tile_pool(name="stage", bufs=6) as spool, \
             tc.tile_pool(name="psum", bufs=2, space="PSUM") as qpool:
            # every DMA (loads + stores) is issued by the sync engine so
            # ACT/DVE stay free for the PSUM drain; loads are ordered by
            # first use (factor chunks before the patch chunk they gate).
            loads = []  # (first_use_pos, order, kind, a, ln)
            for li, (a, ln) in enumerate(zip(pstarts, psizes)):
                loads.append((p_first[li], 1, "p", int(a), ln))
            for li, (a, ln) in enumerate(zip(wstarts, wsizes)):
                loads.append((w_first[li], 0, "w", int(a), ln))
            loads.sort()
            ptiles = []
            wtiles = []
            xtiles = []
            # patch loads + stores on the sync ring; factor loads on the
            # scalar ring so both issue/transfer streams run in parallel
            for (fu, _o, kind, a, ln) in loads:
                if kind == "p":
                    pt = ipool.tile([128, ln * C], _F16, tag=f"p{a}")
                    nc.sync.dma_start(out=pt[:],
                                      in_=patch_d[:, a * C:(a + ln) * C])
                    ptiles.append((a, pt))
                else:
                    wt = ipool.tile([128, ln * 2 * CS], _F16, tag=f"wy{a}")
                    nc.sync.dma_start(
                        out=wt[:],
                        in_=wyab_d[:, a * 2 * CS:(a + ln) * 2 * CS])
                    wtiles.append((a, wt))
                    xt = ipool.tile([128, ln * 2 * CS], _F16, tag=f"wx{a}")
                    nc.sync.dma_start(
                        out=xt[:],
                        in_=wxseg_d[:, a * 2 * CS:(a + ln) * 2 * CS])
                    xtiles.append((a, xt))
            ptiles.sort()
            wtiles.sort()
            xtiles.sort()

            def patch_ap(s):
                li = int(np.searchsorted(pstarts, s, side="right")) - 1
                a, pt = ptiles[li]
                return pt[:, (s - a) * C:(s - a + 1) * C]

            def factor_aps(t0, g):
                """[t0, t0+g) factor views; a batch never spans a load
                chunk because LOAD_SEGS % BG == 0."""
                li = int(np.searchsorted(wstarts, t0, side="right")) - 1
                a, wt = wtiles[li]
                _, xt = xtiles[li]
                wy = wt[:, (t0 - a) * 2 * CS:(t0 - a + g) * 2 * CS]
                wx = xt[:, (t0 - a) * 2 * CS:(t0 - a + g) * 2 * CS]
                return wy, wx

            # batched weight builds, one op per BG consecutive emission
            # positions, alternating DVE / GpSimd
            w2dQ = {}  # t0 -> (tile, g)
            for bi, t0 in enumerate(range(0, T, BG)):
                g = min(BG, T - t0)
                wq = wpool.tile([128, BG * 2 * TT], _F16, tag="w2d")
                wy, wx = factor_aps(t0, g)
                wy_v = wy.rearrange("p (sb i) -> p sb i", i=CS) \
                    .unsqueeze(3).broadcast_to([128, 2 * g, CS, CS])
                wx_v = wx.rearrange("p (sb j) -> p sb j", j=CS) \
                    .unsqueeze(2).broadcast_to([128, 2 * g, CS, CS])
                out_v = wq[:, 0:g * 2 * TT].rearrange(
                    "p (sb i j) -> p sb i j", i=CS, j=CS)
                nbat = (T + BG - 1) // BG
                if bi % 2 == 1 or bi == nbat - 1:
                    nc.gpsimd.tensor_mul(out_v, wy_v, wx_v)
                else:
                    nc.vector.tensor_mul(out_v, wy_v, wx_v)
                w2dQ[t0] = (wq, g)

            stage = None
            ps = None
            chunk_of = {}
            for ci_, (g0, ng, r0, rows) in enumerate(chunks):
                for gi in range(g0, g0 + ng):
                    chunk_of[gi] = ci_
            for gi in gi_order:
                ci_ = chunk_of[gi]
                g0, ng, r0, rows = chunks[ci_]
                if stage is None:
                    stage = spool.tile([128, rows * 2 * TT], _F16,
                                       tag=f"s{rows}")
                # one 4-bank PSUM tile per TWO pairs: blocks
                # [instA-even, instA-odd, instB-even, instB-odd]
                half = gi % 2
                if half == 1 or gi == len(template) - 1:
                    ps = qpool.tile([128, 2048], _F32, tag="ps")
                segs = inst_segs[gi]
                for si, (sl, part0, cnt, pixoff) in enumerate(segs):
                    t = int(pos_of_seg[int(seg_off[gi]) + si])
                    wq, _ = w2dQ[(t // BG) * BG]
                    k = t % BG
                    rhs = wq[:, k * 2 * TT:(k + 1) * 2 * TT]
                    pap = patch_ap(sl)
                    lhsT_e = pap.rearrange("p (c t) -> p t c", t=2)[:, 0, :]
                    lhsT_o = pap.rearrange("p (c t) -> p t c", t=2)[:, 1, :]
                    st = (si == 0)
                    sp = (si == len(segs) - 1)
                    b0 = half * 1024
                    nc.tensor.matmul(ps[:, b0:b0 + 2 * TT], lhsT=lhsT_e,
                                     rhs=rhs, start=st, stop=sp)
                    nc.tensor.matmul(ps[:, b0 + 512:b0 + 512 + 2 * TT],
                                     lhsT=lhsT_o, rhs=rhs, start=st, stop=sp)
                if half == 0:  # group (gi, gi+1) complete -> drain
                    nb = 4 if gi + 1 < len(template) else 2
                    src = ps[:].rearrange("p (b x) -> p b x",
                                          b=4)[:, 0:nb, 0:2 * TT]
                    soff = (gi - g0) * 2 * TT * 2
                    dst = stage[:, soff:soff + nb * 2 * TT].rearrange(
                        "p (b x) -> p b x", b=nb)
                    # alternate drains ~12:20 DVE:ACT so two are in flight
                    if (gi // 2) % 4 == 0:
                        nc.vector.tensor_copy(out=dst, in_=src)
                    else:
                        nc.scalar.copy(out=dst, in_=src)
                if gi == g0:  # close chunk (lowest instance emitted last)
                    nc.sync.dma_start(
                        out=out_d[:, r0 * 2 * TT:(r0 + rows) * 2 * TT],
                        in_=stage[:, 0:rows * 2 * TT])
                    stage = None
    nc.finalize()
    return nc


_NC_CACHE = {}


def _get_program(layout):
    template, S, R, T, inst_segs = layout[:5]
    key = (tuple(template), tuple(tuple(s) for s in inst_segs))
    if key not in _NC_CACHE:
        _NC_CACHE[key] = build_bass_program(layout)
    return _NC_CACHE[key]


# ---------------------------------------------------------------- entry
def _prepare(p2, p3, p4, p5, proposals):
    feats = [np.asarray(p, np.float32) for p in (p2, p3, p4, p5)]
    featcat = np.concatenate(
        [np.ascontiguousarray(f.transpose(0, 2, 3, 1)).reshape(-1, C)
         for f in feats], axis=0)
    featcat16 = featcat.astype(np.float16)
    plans = _plan_proposals(proposals)

    pairs = _pair_overlap(plans)
    template, core_groups = _deal(pairs)
    S, R, inst_segs, g_row, chunks, seg_off = _place(template)
    T = int(seg_off[-1])
    gi_order, pos_of_seg = _emission(template, inst_segs, chunks, seg_off)
    layout = (template, S, R, T, inst_segs, g_row, chunks, seg_off,
              gi_order, pos_of_seg)

    patches = np.zeros((NCORES, 128, S * C), np.float16)
    wyabs = np.zeros((NCORES, 128, T * 2 * CS), np.float16)
    wxsegs = np.zeros((NCORES, 128, T * 2 * CS), np.float16)
    rowmap = []
    for core in range(NCORES):
        patch_c = np.zeros((S, 128, C), np.float16)
        wyab_c = np.zeros((T, 128, 2 * CS), np.float16)
        wxseg_c = np.zeros((T, 128, 2 * CS), np.float16)
        for gi, grp in enumerate(core_groups[core]):
            if grp is None:
                continue
            segs = inst_segs[gi]
            r0 = g_row[gi]
            nu = grp["nu"]
            wy_u = np.zeros((nu, 2 * CS), np.float16)
            wx_u = np.zeros((nu, 2 * CS), np.float16)
            for k, pid in enumerate(grp["pids"]):
                _, wy_px, wx_px = plans[pid]
                p = grp["pos"][k]
                wy_u[p, k * CS:(k + 1) * CS] = wy_px
                wx_u[p, k * CS:(k + 1) * CS] = wx_px
                rowmap.append((core, r0 + k, pid))
            uidx = grp["uidx"]
            for si, (sl, part0, cnt, pixoff) in enumerate(segs):
                take = min(cnt, nu - pixoff)
                if take <= 0:
                    continue
                sel = slice(pixoff, pixoff + take)
                prange = slice(part0, part0 + take)
                t = int(pos_of_seg[int(seg_off[gi]) + si])
                patch_c[sl, prange] = featcat16[uidx[sel]]
                wyab_c[t, prange] = wy_u[sel]
                wxseg_c[t, prange] = wx_u[sel]
        patches[core] = np.ascontiguousarray(
            patch_c.transpose(1, 0, 2)).reshape(128, S * C)
        wyabs[core] = np.ascontiguousarray(
            wyab_c.transpose(1, 0, 2)).reshape(128, T * 2 * CS)
        wxsegs[core] = np.ascontiguousarray(
            wxseg_c.transpose(1, 0, 2)).reshape(128, T * 2 * CS)
    return layout, patches, wyabs, wxsegs, rowmap


def run(p2, p3, p4, p5, proposals, trace=False):
    layout, patches, wyabs, wxsegs, rowmap = _prepare(
        p2, p3, p4, p5, proposals)
    nc = _get_program(layout)
    in_maps = [{"patch": patches[c], "wyab": wyabs[c], "wxseg": wxsegs[c]}
               for c in range(NCORES)]
    res = bass_utils.run_bass_kernel_spmd(
        nc, in_maps, core_ids=list(range(NCORES)), trace=trace)
    out = np.empty((NPROP, C, CS, CS), np.float32)
    done = np.zeros(NPROP, bool)
    for core in range(NCORES):
        R = res.results[core]["out"].shape[1] // (2 * TT)
        # per instance the stage holds [even(prop0,prop1), odd(prop0,prop1)]
        co = res.results[core]["out"].reshape(128, R // 2, 2, 2, TT)
        full = np.empty((R, C, TT), np.float32)
        full[0::2, 0::2] = co[:, :, 0, 0].transpose(1, 0, 2)
        full[0::2, 1::2] = co[:, :, 1, 0].transpose(1, 0, 2)
        full[1::2, 0::2] = co[:, :, 0, 1].transpose(1, 0, 2)
        full[1::2, 1::2] = co[:, :, 1, 1].transpose(1, 0, 2)
        for (c, row, pid) in rowmap:
            if c == core:
                out[pid] = full[row].reshape(C, CS, CS)
                done[pid] = True
    assert done.all(), "some proposals unassigned"
    return out, res


def kernel(p2, p3, p4, p5, proposals):
    out, _res = run(p2, p3, p4, p5, proposals, trace=False)
    return out
